# revision 18
# baseline (speedup 1.0000x reference)
"""ANOVA kernel (order 3) for Trainium2, 8 NeuronCores, pure data parallel.

Reference computation per sample b (x: (B, F, D) fp32):
    out[b] = sum_d e3(x[b, :, d])
where e3 is the 3rd elementary symmetric polynomial over the F=40 field values.

Newton's identities replace the sequential DP over F with power sums:
    p_k[b, d] = sum_f x[b, f, d]^k          (k = 1, 2, 3)
    e3 = (p1^3 - 3*p1*p2 + 2*p3) / 6

Per-core mapping (batch on partitions, 16 groups of 128 batches):
  - gpsimd (SWDGE) casting DMA streams x fp32->fp16 in f-chunks; DMA cost
    scales with the *output* bytes so casting halves the stream time.
  - ScalarE: X2 = Square(X); VectorE: X3 = X * X2 (all fp16).
  - TensorE: power sums via PSUM-accumulating matmuls with scaled identity
    stationaries (p1ps = c*p1, c = 6^(-1/3); p2ps = -3c^2*p2; p3ps = p3/3
    accumulated over f AND d via stride-0 output AP dims kept non-inner).
  - p1 is optionally computed by a pairwise add tree (GPSIMD + DVE halves,
    PE finishing the last 5 fields) to offload the Tensor engine.
  - Tail: t1 = p1ps^2 (ACT), t3 = t1 + p2ps, t4 = t3 * p1ps (DVE), one
    matmul folds sum_d t4 into p3ps, DVE copies PSUM to an SBUF staging
    tile; a single [128, 16] output DMA at the end.
"""

import numpy as np
from contextlib import ExitStack

import concourse.bacc as bacc
import concourse.mybir as mybir
import concourse.tile as tile
from concourse import masks
from concourse.bass_utils import run_bass_kernel_spmd
from bass_rust import add_dep_helper as bass_add_dep

N_CORES = 8
B, F, D = 16384, 40, 64
B_SHARD = B // N_CORES          # 2048 batches per core
GROUPS = B_SHARD // 128         # 16 groups of 128 batches

FP32 = mybir.dt.float32
FP16 = mybir.dt.float16
C1 = 6.0 ** (-1.0 / 3.0)
C2 = -3.0 * 6.0 ** (-2.0 / 3.0)
C3 = 1.0 / 3.0

# (nt, chunk_sizes, p1_mode): p1_mode "tree" offloads p1's reduce to a
# GPSIMD/DVE pairwise add tree; "pe" uses a plain matmul group.
TILES = [
    (4, [4, 6, 10, 20], "tree"),
    (4, [20, 20], "tree"),
    (4, [20, 20], "tree"),
    (2, [20, 20], "pe"),
    (2, [20, 16, 4], "pe"),
]
assert sum(nt for nt, _, _ in TILES) == GROUPS


def _dedupe_ldweights(nc):
    """Remove InstLdweights that reload weights already resident in the PE
    array. Waits/updates of a removed load migrate to the next PE inst."""
    PE = mybir.EngineType.PE
    removed = 0
    for block in nc.m.functions[0].blocks:
        insts = block.instructions
        cur_sig = None
        pending_sync = []
        keep = []
        for inst in insts:
            nm = type(inst).__name__
            if pending_sync and getattr(inst, "engine", None) == PE:
                si = inst.sync_info
                if si is None:
                    si = mybir.SyncInfo(on_wait=[], on_update=[])
                    inst.sync_info = si
                for psi in pending_sync:
                    si.on_wait = list(psi.on_wait) + list(si.on_wait)
                    si.on_update = list(si.on_update) + list(psi.on_update)
                pending_sync = []
            if nm == "InstMatmult":
                wap = inst.ins[1]
                if str(wap.dtype) in ("dt.float32", "dt.float32r",
                                      "float32", "float32r"):
                    cur_sig = None
            elif nm == "InstLdweights":
                wap = inst.ins[0]
                sig = (str(wap.memref), wap.offset, str(wap.ap), str(wap.dtype))
                if sig == cur_sig:
                    si = inst.sync_info
                    if si is not None and (si.on_wait or si.on_update):
                        pending_sync.append(si)
                    removed += 1
                    continue
                cur_sig = sig
            keep.append(inst)
        assert not pending_sync, "dangling sync from removed trailing ldweights"
        block.instructions = keep
    return removed


def build_nc():
    nc = bacc.Bacc("TRN2", target_bir_lowering=False, debug=False,
                   num_devices=N_CORES, dynamic_dma_scratch_size=65536)
    x = nc.dram_tensor("x", [B_SHARD, F, D], FP32, kind="ExternalInput")
    out = nc.dram_tensor("out", [128, GROUPS], FP32, kind="ExternalOutput")

    x_r = x.rearrange("(g p) f d -> p g f d", p=128)

    with tile.TileContext(nc) as tc, ExitStack() as ctx:
        const = ctx.enter_context(tc.tile_pool(name="const", bufs=1))
        xp = ctx.enter_context(tc.tile_pool(name="xp", bufs=3))
        x2p = ctx.enter_context(tc.tile_pool(name="x2p", bufs=2))
        x3p = ctx.enter_context(tc.tile_pool(name="x3p", bufs=1))
        gp = ctx.enter_context(tc.tile_pool(name="gp", bufs=1))
        tp = ctx.enter_context(tc.tile_pool(name="tp", bufs=2))
        stg = ctx.enter_context(tc.tile_pool(name="stg", bufs=1))
        psum = ctx.enter_context(tc.tile_pool(name="psum", bufs=2, space="PSUM"))

        ident = const.tile([128, 128], FP16)
        masks.make_identity(nc, ident[:])
        ident_c1 = const.tile([128, 128], FP16)
        nc.vector.tensor_scalar_mul(ident_c1[:], ident[:], C1)
        ident_c2 = const.tile([128, 128], FP16)
        nc.vector.tensor_scalar_mul(ident_c2[:], ident[:], C2)
        ident_c3 = const.tile([128, 128], FP16)
        nc.vector.tensor_scalar_mul(ident_c3[:], ident[:], C3)

        stage = stg.tile([128, GROUPS], FP32, tag="stage")

        pending = {}   # i -> (g0, nt, mode, X, p1ps, p2ps, p3ps, tree tiles)

        def bcast_r(ps, r):
            ap = ps[:]
            return ap.__replace__(ap=[ap.ap[0], ap.ap[1], [0, r], ap.ap[2]])

        def bcast_rd(ps, r):
            ap = ps[:]
            return ap.__replace__(ap=[ap.ap[0], [0, r], [0, D], ap.ap[1]])

        def mm_group(out_ap_fn, ps, ident_ap, src, nf, stop_last=True,
                     reorder=False, start=True, f_start=0):
            for f0 in range(f_start, nf, 2):
                r = min(2, nf - f0)
                mv = src[:, :, f0:f0 + r, :]
                if reorder:
                    mv = mv.rearrange("p g f d -> p f d g")
                nc.tensor.matmul(out_ap_fn(ps, r), lhsT=ident_ap, rhs=mv,
                                 start=(start and f0 == 0),
                                 stop=(stop_last and f0 + r >= nf),
                                 skip_group_check=True)

        def emit_tree_l1(i, order_after=None):
            """Level 1 (40->20) of tile i's p1 add tree on GPSIMD in 5-field
            slabs (ordering edges keep them behind the next tile's DMA gens
            in the in-order Pool queue)."""
            (g0, nt, mode, X, G, p1ps, p2ps, p3ps) = pending[i]
            G = gp.tile([128, 4, 20, D], FP16, tag="G", name="G")[:, 0:nt]
            for s0 in range(0, 20, 5):
                add = nc.gpsimd.tensor_add(G[:, :, s0:s0 + 5, :],
                                           X[:, :, s0:s0 + 5, :],
                                           X[:, :, s0 + 20:s0 + 25, :])
                if order_after is not None:
                    bass_add_dep(add.ins, order_after.ins, sync=False,
                                 reason="tree slab after next tile's DMA gens")
            pending[i] = (g0, nt, mode, X, G, p1ps, p2ps, p3ps)

        def emit_tree_mid(i):
            """Levels 2-3 of tile i's p1 add tree (emitted in tile i+1)."""
            (g0, nt, mode, X, G, p1ps, p2ps, p3ps) = pending[i]
            G2 = gp.tile([128, 4, 10, D], FP16, tag="G2", name="G2")[:, 0:nt]
            nc.vector.tensor_add(G2[:], G[:, :, 0:10, :], G[:, :, 10:20, :])
            G3 = gp.tile([128, 4, 5, D], FP16, tag="G3", name="G3")[:, 0:nt]
            nc.vector.tensor_add(G3[:], G2[:, :, 0:5, :], G2[:, :, 5:10, :])
            pending[i] = (g0, nt, mode, X, G3, p1ps, p2ps, p3ps)

        def emit_tail(i):
            """Finish tile i: p1 (red5 or nothing), e3 combine, staging."""
            (g0, nt, mode, X, G3, p1ps, p2ps, p3ps) = pending.pop(i)
            if mode == "tree":
                mm_group(bcast_r, p1ps, ident_c1[:], G3, 5)
            t1 = tp.tile([128, 4, D], FP16, tag="t1", name="t1")[:, 0:nt]
            nc.scalar.square(t1[:], p1ps[:])               # c^2 p1^2
            t3 = tp.tile([128, 4, D], FP16, tag="t3", name="t3")[:, 0:nt]
            nc.vector.tensor_add(t3[:], t1[:], p2ps[:])    # c^2(p1^2-3p2)
            t4 = tp.tile([128, 4, D], FP16, tag="t4", name="t4")[:, 0:nt]
            mul = nc.vector.tensor_mul(t4[:], t3[:], p1ps[:])
            # close the p3 group: p3ps += sum_d t4 (stride-0 dim not inner)
            ap = p3ps[:]
            bd = ap.__replace__(ap=[ap.ap[0], [0, D], ap.ap[1]])
            nc.tensor.matmul(bd, lhsT=ident[:],
                             rhs=t4[:].rearrange("p g d -> p d g"),
                             start=False, stop=True, skip_group_check=True)
            cp = nc.vector.tensor_copy(stage[:, g0:g0 + nt], p3ps[:])
            return mul, cp

        g0 = 0
        for i, (nt, sizes, mode) in enumerate(TILES):
            X = xp.tile([128, 4, F, D], FP16, tag="X", name="X")[:, 0:nt]
            X2 = x2p.tile([128, 4, F, D], FP16, tag="X2", name="X2")[:, 0:nt]
            X3 = x3p.tile([128, 4, F, D], FP16, tag="X3", name="X3")[:, 0:nt]
            assert sum(sizes) == F
            bounds = [0]
            for s in sizes:
                bounds.append(bounds[-1] + s)
            chunks = [slice(a, b) for a, b in zip(bounds[:-1], bounds[1:])]
            last_dma = None
            for fs in chunks:
                last_dma = nc.gpsimd.dma_start(X[:, :, fs, :],
                                               x_r[:, g0:g0 + nt, fs, :])
            prev_tree = (i >= 1 and pending.get(i - 1) is not None
                         and pending[i - 1][2] == "tree")
            if prev_tree:
                emit_tree_l1(i - 1, order_after=last_dma)
            for fs in chunks:
                nc.scalar.square(X2[:, :, fs, :], X[:, :, fs, :])

            p1ps = psum.tile([128, 4, D], FP32, tag="p1ps", name="p1ps")[:, 0:nt]
            p2ps = psum.tile([128, 4, D], FP32, tag="p2ps", name="p2ps")[:, 0:nt]
            p3ps = psum.tile([128, 4], FP32, tag="p3ps", name="p3ps")[:, 0:nt]

            # per-chunk: X3 mult, then this chunk's p2/p3 matmuls (lets X3
            # live in a single buffer and keeps PE fed chunk-by-chunk)
            prev_ops = None
            nch = len(chunks)
            for ci, fs in enumerate(chunks):
                if ci == 1 and prev_tree:
                    emit_tree_mid(i - 1)
                if ci == nch - 1 and i >= 1 and (i - 1) in pending:
                    prev_ops = emit_tail(i - 1)
                mul = nc.vector.tensor_mul(X3[:, :, fs, :], X[:, :, fs, :],
                                           X2[:, :, fs, :])
                if prev_ops is not None and ci == nch - 1:
                    bass_add_dep(mul.ins, prev_ops[0].ins, sync=False,
                                 reason="drain prev-tile combine first")
                if mode != "tree":
                    mm_group(bcast_r, p1ps, ident_c1[:], X, fs.stop,
                             f_start=fs.start, start=(fs.start == 0),
                             stop_last=(fs.stop == F))
                mm_group(bcast_r, p2ps, ident_c2[:], X2, fs.stop,
                         f_start=fs.start, start=(fs.start == 0),
                         stop_last=(fs.stop == F))
                mm_group(bcast_rd, p3ps, ident_c3[:], X3, fs.stop,
                         f_start=fs.start, start=(fs.start == 0),
                         stop_last=False, reorder=True)

            pending[i] = (g0, nt, mode, X, G := None, p1ps, p2ps, p3ps)
            g0 += nt

        last = len(TILES) - 1
        if pending[last][2] == "tree":
            emit_tree_l1(last)
            emit_tree_mid(last)
        emit_tail(last)
        nc.sync.dma_start(out[:], stage[:])

    _dedupe_ldweights(nc)
    nc.finalize()
    return nc


_NC_CACHE = None


def _get_nc():
    global _NC_CACHE
    if _NC_CACHE is None:
        _NC_CACHE = build_nc()
    return _NC_CACHE


def run(x: np.ndarray, **spmd_kwargs):
    """Run on 8 cores; returns (out (B,1) fp32, BassKernelResults)."""
    assert x.shape == (B, F, D), x.shape
    x = np.ascontiguousarray(x, dtype=np.float32)
    nc = _get_nc()
    in_maps = [{"x": x[i * B_SHARD:(i + 1) * B_SHARD]} for i in range(N_CORES)]
    res = run_bass_kernel_spmd(nc, in_maps, core_ids=list(range(N_CORES)),
                               **spmd_kwargs)
    outs = []
    for i in range(N_CORES):
        o = res.results[i]["out"]          # [128, GROUPS], out[p, g] = b g*128+p
        outs.append(o.T.reshape(B_SHARD, 1))
    return np.concatenate(outs, axis=0), res


def kernel(x: np.ndarray) -> np.ndarray:
    out, _ = run(x)
    return out


if __name__ == "__main__":
    rng = np.random.default_rng(0)
    x = rng.standard_normal((B, F, D)).astype(np.float32)
    out = kernel(x)
    print("out", out.shape, out.dtype, out[:4, 0])


# revision 19
# speedup vs baseline: 1.1673x; 1.1673x over previous
"""ANOVA kernel (order 3) for Trainium2, 8 NeuronCores, pure data parallel.

Reference computation per sample b (x: (B, F, D) fp32):
    out[b] = sum_d e3(x[b, :, d])
where e3 is the 3rd elementary symmetric polynomial over the F=40 field values.

Newton's identities replace the sequential DP over F with power sums:
    p_k[b, d] = sum_f x[b, f, d]^k          (k = 1, 2, 3)
    e3 = (p1^3 - 3*p1*p2 + 2*p3) / 6

Per-core mapping (batch on partitions, 16 groups of 128 batches):
  - gpsimd (SWDGE) casting DMA streams x fp32->fp16 in f-chunks; DMA cost
    scales with the *output* bytes so casting halves the stream time.
  - ScalarE: X2 = Square(X); VectorE: X3 = X * X2 (all fp16).
  - TensorE: power sums via PSUM-accumulating matmuls with scaled identity
    stationaries (p1ps = c*p1, c = 6^(-1/3); p2ps = -3c^2*p2; p3ps = p3/3
    accumulated over f AND d via stride-0 output AP dims kept non-inner).
  - p1 is optionally computed by a pairwise add tree (GPSIMD + DVE halves,
    PE finishing the last 5 fields) to offload the Tensor engine.
  - Tail: t1 = p1ps^2 (ACT), t3 = t1 + p2ps, t4 = t3 * p1ps (DVE), one
    matmul folds sum_d t4 into p3ps, DVE copies PSUM to an SBUF staging
    tile; a single [128, 16] output DMA at the end.
"""

import numpy as np
from contextlib import ExitStack

import concourse.bacc as bacc
import concourse.mybir as mybir
import concourse.tile as tile
from concourse import masks
from concourse.bass_utils import run_bass_kernel_spmd
from bass_rust import add_dep_helper as bass_add_dep

N_CORES = 8
B, F, D = 16384, 40, 64
B_SHARD = B // N_CORES          # 2048 batches per core
GROUPS = B_SHARD // 128         # 16 groups of 128 batches

FP32 = mybir.dt.float32
FP16 = mybir.dt.float16
C1 = 6.0 ** (-1.0 / 3.0)
C2 = -3.0 * 6.0 ** (-2.0 / 3.0)
C3 = 1.0 / 3.0

# (nt, chunk_sizes, p1_mode): p1_mode "tree" offloads p1's reduce to a
# GPSIMD/DVE pairwise add tree; "pe" uses a plain matmul group.
TILES = [
    (4, [4, 6, 10, 20], "tree"),
    (4, [20, 20], "tree"),
    (4, [20, 20], "tree"),
    (2, [20, 20], "pe"),
    (2, [20, 16, 4], "pe"),
]
assert sum(nt for nt, _, _ in TILES) == GROUPS


def _dedupe_ldweights(nc):
    """Remove InstLdweights that reload weights already resident in the PE
    array. Waits/updates of a removed load migrate to the next PE inst."""
    PE = mybir.EngineType.PE
    removed = 0
    for block in nc.m.functions[0].blocks:
        insts = block.instructions
        cur_sig = None
        pending_sync = []
        keep = []
        for inst in insts:
            nm = type(inst).__name__
            if pending_sync and getattr(inst, "engine", None) == PE:
                si = inst.sync_info
                if si is None:
                    si = mybir.SyncInfo(on_wait=[], on_update=[])
                    inst.sync_info = si
                for psi in pending_sync:
                    si.on_wait = list(psi.on_wait) + list(si.on_wait)
                    si.on_update = list(si.on_update) + list(psi.on_update)
                pending_sync = []
            if nm == "InstMatmult":
                wap = inst.ins[1]
                if str(wap.dtype) in ("dt.float32", "dt.float32r",
                                      "float32", "float32r"):
                    cur_sig = None
            elif nm == "InstLdweights":
                wap = inst.ins[0]
                sig = (str(wap.memref), wap.offset, str(wap.ap), str(wap.dtype))
                if sig == cur_sig:
                    si = inst.sync_info
                    if si is not None and (si.on_wait or si.on_update):
                        pending_sync.append(si)
                    removed += 1
                    continue
                cur_sig = sig
            keep.append(inst)
        assert not pending_sync, "dangling sync from removed trailing ldweights"
        block.instructions = keep
    return removed


def build_nc():
    nc = bacc.Bacc("TRN2", target_bir_lowering=False, debug=False,
                   num_devices=N_CORES, dynamic_dma_scratch_size=65536)
    x = nc.dram_tensor("x", [B_SHARD, F, D], FP32, kind="ExternalInput")
    out = nc.dram_tensor("out", [128, GROUPS], FP32, kind="ExternalOutput")

    x_r = x.rearrange("(g p) f d -> p g f d", p=128)

    with tile.TileContext(nc) as tc, ExitStack() as ctx:
        const = ctx.enter_context(tc.tile_pool(name="const", bufs=1))
        xp = ctx.enter_context(tc.tile_pool(name="xp", bufs=3))
        x2p = ctx.enter_context(tc.tile_pool(name="x2p", bufs=2))
        x3p = ctx.enter_context(tc.tile_pool(name="x3p", bufs=1))
        gp = ctx.enter_context(tc.tile_pool(name="gp", bufs=1))
        tp = ctx.enter_context(tc.tile_pool(name="tp", bufs=2))
        stg = ctx.enter_context(tc.tile_pool(name="stg", bufs=1))
        psum = ctx.enter_context(tc.tile_pool(name="psum", bufs=2, space="PSUM"))

        ident = const.tile([128, 128], FP16)
        masks.make_identity(nc, ident[:])
        ident_c1 = const.tile([128, 128], FP16)
        nc.vector.tensor_scalar_mul(ident_c1[:], ident[:], C1)
        ident_c2 = const.tile([128, 128], FP16)
        nc.vector.tensor_scalar_mul(ident_c2[:], ident[:], C2)
        ident_c3 = const.tile([128, 128], FP16)
        nc.vector.tensor_scalar_mul(ident_c3[:], ident[:], C3)

        stage = stg.tile([128, GROUPS], FP32, tag="stage")

        pending = {}   # i -> (g0, nt, mode, X, p1ps, p2ps, p3ps, tree tiles)

        def bcast_r(ps, r):
            ap = ps[:]
            return ap.__replace__(ap=[ap.ap[0], ap.ap[1], [0, r], ap.ap[2]])

        def bcast_rd(ps, r):
            ap = ps[:]
            return ap.__replace__(ap=[ap.ap[0], [0, r], [0, D], ap.ap[1]])

        def mm_group(out_ap_fn, ps, ident_ap, src, nf, stop_last=True,
                     reorder=False, start=True, f_start=0):
            for f0 in range(f_start, nf, 2):
                r = min(2, nf - f0)
                mv = src[:, :, f0:f0 + r, :]
                if reorder:
                    mv = mv.rearrange("p g f d -> p f d g")
                nc.tensor.matmul(out_ap_fn(ps, r), lhsT=ident_ap, rhs=mv,
                                 start=(start and f0 == 0),
                                 stop=(stop_last and f0 + r >= nf),
                                 skip_group_check=True)

        def emit_tree_l1(i, order_after=None):
            """Level 1 (40->20) of tile i's p1 add tree, on DVE."""
            (g0, nt, mode, X, G, p1ps, p2ps, p3ps) = pending[i]
            G = gp.tile([128, 4, 20, D], FP16, tag="G", name="G")[:, 0:nt]
            nc.vector.tensor_add(G[:, :, 0:10, :], X[:, :, 0:10, :],
                                 X[:, :, 20:30, :])
            nc.vector.tensor_add(G[:, :, 10:20, :], X[:, :, 10:20, :],
                                 X[:, :, 30:40, :])
            pending[i] = (g0, nt, mode, X, G, p1ps, p2ps, p3ps)

        def emit_tree_mid(i):
            """Levels 2-3 of tile i's p1 add tree (emitted in tile i+1)."""
            (g0, nt, mode, X, G, p1ps, p2ps, p3ps) = pending[i]
            G2 = gp.tile([128, 4, 10, D], FP16, tag="G2", name="G2")[:, 0:nt]
            nc.vector.tensor_add(G2[:], G[:, :, 0:10, :], G[:, :, 10:20, :])
            G3 = gp.tile([128, 4, 5, D], FP16, tag="G3", name="G3")[:, 0:nt]
            nc.vector.tensor_add(G3[:], G2[:, :, 0:5, :], G2[:, :, 5:10, :])
            pending[i] = (g0, nt, mode, X, G3, p1ps, p2ps, p3ps)

        def emit_tail(i):
            """Finish tile i: p1 (red5 or nothing), e3 combine, staging."""
            (g0, nt, mode, X, G3, p1ps, p2ps, p3ps) = pending.pop(i)
            if mode == "tree":
                mm_group(bcast_r, p1ps, ident_c1[:], G3, 5)
            t1 = tp.tile([128, 4, D], FP16, tag="t1", name="t1")[:, 0:nt]
            nc.scalar.square(t1[:], p1ps[:])               # c^2 p1^2
            t3 = tp.tile([128, 4, D], FP16, tag="t3", name="t3")[:, 0:nt]
            nc.vector.tensor_add(t3[:], t1[:], p2ps[:])    # c^2(p1^2-3p2)
            t4 = tp.tile([128, 4, D], FP16, tag="t4", name="t4")[:, 0:nt]
            mul = nc.vector.tensor_mul(t4[:], t3[:], p1ps[:])
            # close the p3 group: p3ps += sum_d t4 (stride-0 dim not inner)
            ap = p3ps[:]
            bd = ap.__replace__(ap=[ap.ap[0], [0, D], ap.ap[1]])
            nc.tensor.matmul(bd, lhsT=ident[:],
                             rhs=t4[:].rearrange("p g d -> p d g"),
                             start=False, stop=True, skip_group_check=True)
            cp = nc.vector.tensor_copy(stage[:, g0:g0 + nt], p3ps[:])
            return mul, cp

        g0 = 0
        for i, (nt, sizes, mode) in enumerate(TILES):
            X = xp.tile([128, 4, F, D], FP16, tag="X", name="X")[:, 0:nt]
            X2 = x2p.tile([128, 4, F, D], FP16, tag="X2", name="X2")[:, 0:nt]
            X3 = x3p.tile([128, 4, F, D], FP16, tag="X3", name="X3")[:, 0:nt]
            assert sum(sizes) == F
            bounds = [0]
            for s in sizes:
                bounds.append(bounds[-1] + s)
            chunks = [slice(a, b) for a, b in zip(bounds[:-1], bounds[1:])]
            last_dma = None
            for fs in chunks:
                last_dma = nc.gpsimd.dma_start(X[:, :, fs, :],
                                               x_r[:, g0:g0 + nt, fs, :])
            prev_tree = (i >= 1 and pending.get(i - 1) is not None
                         and pending[i - 1][2] == "tree")
            if prev_tree:
                emit_tree_l1(i - 1, order_after=last_dma)
            for fs in chunks:
                nc.scalar.square(X2[:, :, fs, :], X[:, :, fs, :])

            p1ps = psum.tile([128, 4, D], FP32, tag="p1ps", name="p1ps")[:, 0:nt]
            p2ps = psum.tile([128, 4, D], FP32, tag="p2ps", name="p2ps")[:, 0:nt]
            p3ps = psum.tile([128, 4], FP32, tag="p3ps", name="p3ps")[:, 0:nt]

            # per-chunk: X3 mult, then this chunk's p2/p3 matmuls (lets X3
            # live in a single buffer and keeps PE fed chunk-by-chunk)
            prev_ops = None
            nch = len(chunks)
            for ci, fs in enumerate(chunks):
                if ci == 1 and prev_tree:
                    emit_tree_mid(i - 1)
                if ci == nch - 1 and i >= 1 and (i - 1) in pending:
                    prev_ops = emit_tail(i - 1)
                mul = nc.vector.tensor_mul(X3[:, :, fs, :], X[:, :, fs, :],
                                           X2[:, :, fs, :])
                if prev_ops is not None and ci == nch - 1:
                    bass_add_dep(mul.ins, prev_ops[0].ins, sync=False,
                                 reason="drain prev-tile combine first")
                if mode != "tree":
                    mm_group(bcast_r, p1ps, ident_c1[:], X, fs.stop,
                             f_start=fs.start, start=(fs.start == 0),
                             stop_last=(fs.stop == F))
                mm_group(bcast_r, p2ps, ident_c2[:], X2, fs.stop,
                         f_start=fs.start, start=(fs.start == 0),
                         stop_last=(fs.stop == F))
                mm_group(bcast_rd, p3ps, ident_c3[:], X3, fs.stop,
                         f_start=fs.start, start=(fs.start == 0),
                         stop_last=False, reorder=True)

            pending[i] = (g0, nt, mode, X, G := None, p1ps, p2ps, p3ps)
            g0 += nt

        last = len(TILES) - 1
        if pending[last][2] == "tree":
            emit_tree_l1(last)
            emit_tree_mid(last)
        emit_tail(last)
        nc.sync.dma_start(out[:], stage[:])

    _dedupe_ldweights(nc)
    nc.finalize()
    return nc


_NC_CACHE = None


def _get_nc():
    global _NC_CACHE
    if _NC_CACHE is None:
        _NC_CACHE = build_nc()
    return _NC_CACHE


def run(x: np.ndarray, **spmd_kwargs):
    """Run on 8 cores; returns (out (B,1) fp32, BassKernelResults)."""
    assert x.shape == (B, F, D), x.shape
    x = np.ascontiguousarray(x, dtype=np.float32)
    nc = _get_nc()
    in_maps = [{"x": x[i * B_SHARD:(i + 1) * B_SHARD]} for i in range(N_CORES)]
    res = run_bass_kernel_spmd(nc, in_maps, core_ids=list(range(N_CORES)),
                               **spmd_kwargs)
    outs = []
    for i in range(N_CORES):
        o = res.results[i]["out"]          # [128, GROUPS], out[p, g] = b g*128+p
        outs.append(o.T.reshape(B_SHARD, 1))
    return np.concatenate(outs, axis=0), res


def kernel(x: np.ndarray) -> np.ndarray:
    out, _ = run(x)
    return out


if __name__ == "__main__":
    rng = np.random.default_rng(0)
    x = rng.standard_normal((B, F, D)).astype(np.float32)
    out = kernel(x)
    print("out", out.shape, out.dtype, out[:4, 0])


# revision 20
# speedup vs baseline: 1.1960x; 1.0246x over previous
"""ANOVA kernel (order 3) for Trainium2, 8 NeuronCores, pure data parallel.

Reference computation per sample b (x: (B, F, D) fp32):
    out[b] = sum_d e3(x[b, :, d])
where e3 is the 3rd elementary symmetric polynomial over the F=40 field values.

Newton's identities replace the sequential DP over F with power sums:
    p_k[b, d] = sum_f x[b, f, d]^k          (k = 1, 2, 3)
    e3 = (p1^3 - 3*p1*p2 + 2*p3) / 6

Per-core mapping (batch on partitions, 16 groups of 128 batches):
  - gpsimd (SWDGE) casting DMA streams x fp32->fp16 in f-chunks; DMA cost
    scales with the *output* bytes so casting halves the stream time.
  - ScalarE: X2 = Square(X); VectorE: X3 = X * X2 (all fp16).
  - TensorE: power sums via PSUM-accumulating matmuls with scaled identity
    stationaries (p1ps = c*p1, c = 6^(-1/3); p2ps = -3c^2*p2; p3ps = p3/3
    accumulated over f AND d via stride-0 output AP dims kept non-inner).
  - p1 is optionally computed by a pairwise add tree (GPSIMD + DVE halves,
    PE finishing the last 5 fields) to offload the Tensor engine.
  - Tail: t1 = p1ps^2 (ACT), t3 = t1 + p2ps, t4 = t3 * p1ps (DVE), one
    matmul folds sum_d t4 into p3ps, DVE copies PSUM to an SBUF staging
    tile; a single [128, 16] output DMA at the end.
"""

import numpy as np
from contextlib import ExitStack

import concourse.bacc as bacc
import concourse.mybir as mybir
import concourse.tile as tile
from concourse import masks
from concourse.bass_utils import run_bass_kernel_spmd
from bass_rust import add_dep_helper as bass_add_dep

N_CORES = 8
B, F, D = 16384, 40, 64
B_SHARD = B // N_CORES          # 2048 batches per core
GROUPS = B_SHARD // 128         # 16 groups of 128 batches

FP32 = mybir.dt.float32
FP16 = mybir.dt.float16
C1 = 6.0 ** (-1.0 / 3.0)
C2 = -3.0 * 6.0 ** (-2.0 / 3.0)
C3 = 1.0 / 3.0

# (nt, chunk_sizes, p1_mode): p1_mode "tree" offloads p1's reduce to a
# GPSIMD/DVE pairwise add tree; "pe" uses a plain matmul group.
TILES = [
    (4, [4, 6, 10, 20], "tree"),
    (4, [20, 20], "tree"),
    (4, [20, 20], "tree"),
    (2, [20, 20], "pe"),
    (2, [20, 16, 4], "pe"),
]
assert sum(nt for nt, _, _ in TILES) == GROUPS


def _dedupe_ldweights(nc):
    """Remove InstLdweights that reload weights already resident in the PE
    array. Waits/updates of a removed load migrate to the next PE inst."""
    PE = mybir.EngineType.PE
    removed = 0
    for block in nc.m.functions[0].blocks:
        insts = block.instructions
        cur_sig = None
        pending_sync = []
        keep = []
        for inst in insts:
            nm = type(inst).__name__
            if pending_sync and getattr(inst, "engine", None) == PE:
                si = inst.sync_info
                if si is None:
                    si = mybir.SyncInfo(on_wait=[], on_update=[])
                    inst.sync_info = si
                for psi in pending_sync:
                    si.on_wait = list(psi.on_wait) + list(si.on_wait)
                    si.on_update = list(si.on_update) + list(psi.on_update)
                pending_sync = []
            if nm == "InstMatmult":
                wap = inst.ins[1]
                if str(wap.dtype) in ("dt.float32", "dt.float32r",
                                      "float32", "float32r"):
                    cur_sig = None
            elif nm == "InstLdweights":
                wap = inst.ins[0]
                sig = (str(wap.memref), wap.offset, str(wap.ap), str(wap.dtype))
                if sig == cur_sig:
                    si = inst.sync_info
                    if si is not None and (si.on_wait or si.on_update):
                        pending_sync.append(si)
                    removed += 1
                    continue
                cur_sig = sig
            keep.append(inst)
        assert not pending_sync, "dangling sync from removed trailing ldweights"
        block.instructions = keep
    return removed


def build_nc():
    nc = bacc.Bacc("TRN2", target_bir_lowering=False, debug=False,
                   num_devices=N_CORES, dynamic_dma_scratch_size=65536)
    x = nc.dram_tensor("x", [B_SHARD, F, D], FP32, kind="ExternalInput")
    out = nc.dram_tensor("out", [128, GROUPS], FP32, kind="ExternalOutput")

    x_r = x.rearrange("(g p) f d -> p g f d", p=128)

    with tile.TileContext(nc) as tc, ExitStack() as ctx:
        const = ctx.enter_context(tc.tile_pool(name="const", bufs=1))
        xp = ctx.enter_context(tc.tile_pool(name="xp", bufs=3))
        x2p = ctx.enter_context(tc.tile_pool(name="x2p", bufs=2))
        x3p = ctx.enter_context(tc.tile_pool(name="x3p", bufs=1))
        gp = ctx.enter_context(tc.tile_pool(name="gp", bufs=1))
        tp = ctx.enter_context(tc.tile_pool(name="tp", bufs=2))
        stg = ctx.enter_context(tc.tile_pool(name="stg", bufs=1))
        psum = ctx.enter_context(tc.tile_pool(name="psum", bufs=2, space="PSUM"))

        ident = const.tile([128, 128], FP16)
        masks.make_identity(nc, ident[:])
        ident_c1 = const.tile([128, 128], FP16)
        nc.vector.tensor_scalar_mul(ident_c1[:], ident[:], C1)
        ident_c2 = const.tile([128, 128], FP16)
        nc.vector.tensor_scalar_mul(ident_c2[:], ident[:], C2)
        ident_c3 = const.tile([128, 128], FP16)
        nc.vector.tensor_scalar_mul(ident_c3[:], ident[:], C3)

        stage = stg.tile([128, GROUPS], FP32, tag="stage")

        pending = {}   # i -> (g0, nt, mode, X, p1ps, p2ps, p3ps, tree tiles)

        def bcast_r(ps, r):
            ap = ps[:]
            return ap.__replace__(ap=[ap.ap[0], ap.ap[1], [0, r], ap.ap[2]])

        def bcast_rd(ps, r):
            ap = ps[:]
            return ap.__replace__(ap=[ap.ap[0], [0, r], [0, D], ap.ap[1]])

        def mm_group(out_ap_fn, ps, ident_ap, src, nf, stop_last=True,
                     reorder=False, start=True, f_start=0):
            for f0 in range(f_start, nf, 2):
                r = min(2, nf - f0)
                mv = src[:, :, f0:f0 + r, :]
                if reorder:
                    mv = mv.rearrange("p g f d -> p f d g")
                nc.tensor.matmul(out_ap_fn(ps, r), lhsT=ident_ap, rhs=mv,
                                 start=(start and f0 == 0),
                                 stop=(stop_last and f0 + r >= nf),
                                 skip_group_check=True)

        def emit_tree_l1(i, order_after=None):
            """Level 1 (40->20) of tile i's p1 add tree, on DVE."""
            (g0, nt, mode, X, G, p1ps, p2ps, p3ps) = pending[i]
            G = gp.tile([128, 4, 20, D], FP16, tag="G", name="G")[:, 0:nt]
            nc.vector.tensor_add(G[:, :, 0:10, :], X[:, :, 0:10, :],
                                 X[:, :, 20:30, :])
            nc.vector.tensor_add(G[:, :, 10:20, :], X[:, :, 10:20, :],
                                 X[:, :, 30:40, :])
            pending[i] = (g0, nt, mode, X, G, p1ps, p2ps, p3ps)

        def emit_tree_mid(i):
            """Levels 2-3 of tile i's p1 add tree (emitted in tile i+1)."""
            (g0, nt, mode, X, G, p1ps, p2ps, p3ps) = pending[i]
            G2 = gp.tile([128, 4, 10, D], FP16, tag="G2", name="G2")[:, 0:nt]
            nc.vector.tensor_add(G2[:], G[:, :, 0:10, :], G[:, :, 10:20, :])
            G3 = gp.tile([128, 4, 5, D], FP16, tag="G3", name="G3")[:, 0:nt]
            nc.vector.tensor_add(G3[:], G2[:, :, 0:5, :], G2[:, :, 5:10, :])
            pending[i] = (g0, nt, mode, X, G3, p1ps, p2ps, p3ps)

        def emit_tail(i):
            """Finish tile i: p1 (red5 or nothing), e3 combine, staging."""
            (g0, nt, mode, X, G3, p1ps, p2ps, p3ps) = pending.pop(i)
            if mode == "tree":
                mm_group(bcast_r, p1ps, ident_c1[:], G3, 5)
            t1 = tp.tile([128, 4, D], FP16, tag="t1", name="t1")[:, 0:nt]
            nc.scalar.square(t1[:], p1ps[:])               # c^2 p1^2
            t3 = tp.tile([128, 4, D], FP16, tag="t3", name="t3")[:, 0:nt]
            nc.vector.tensor_add(t3[:], t1[:], p2ps[:])    # c^2(p1^2-3p2)
            t4 = tp.tile([128, 4, D], FP16, tag="t4", name="t4")[:, 0:nt]
            mul = nc.vector.tensor_mul(t4[:], t3[:], p1ps[:])
            # close the p3 group: p3ps += sum_d t4 (stride-0 dim not inner)
            ap = p3ps[:]
            bd = ap.__replace__(ap=[ap.ap[0], [0, D], ap.ap[1]])
            nc.tensor.matmul(bd, lhsT=ident[:],
                             rhs=t4[:].rearrange("p g d -> p d g"),
                             start=False, stop=True, skip_group_check=True)
            cp = nc.vector.tensor_copy(stage[:, g0:g0 + nt], p3ps[:])
            return mul, cp

        g0 = 0
        for i, (nt, sizes, mode) in enumerate(TILES):
            X = xp.tile([128, 4, F, D], FP16, tag="X", name="X")[:, 0:nt]
            X2 = x2p.tile([128, 4, F, D], FP16, tag="X2", name="X2")[:, 0:nt]
            X3 = x3p.tile([128, 4, F, D], FP16, tag="X3", name="X3")[:, 0:nt]
            assert sum(sizes) == F
            bounds = [0]
            for s in sizes:
                bounds.append(bounds[-1] + s)
            chunks = [slice(a, b) for a, b in zip(bounds[:-1], bounds[1:])]
            last_dma = None
            for fs in chunks:
                last_dma = nc.gpsimd.dma_start(X[:, :, fs, :],
                                               x_r[:, g0:g0 + nt, fs, :])
            prev_tree = (i >= 1 and pending.get(i - 1) is not None
                         and pending[i - 1][2] == "tree")
            if prev_tree:
                emit_tree_l1(i - 1, order_after=last_dma)
            # compute in <=10-field slices (latency) over >=20-field DMA
            # chunks (fewer SWDGE descriptor gens)
            csl = []
            for fs in chunks:
                a = fs.start
                while a < fs.stop:
                    b = min(a + 10, fs.stop)
                    csl.append(slice(a, b))
                    a = b
            for fs in csl:
                nc.scalar.square(X2[:, :, fs, :], X[:, :, fs, :])

            p1ps = psum.tile([128, 4, D], FP32, tag="p1ps", name="p1ps")[:, 0:nt]
            p2ps = psum.tile([128, 4, D], FP32, tag="p2ps", name="p2ps")[:, 0:nt]
            p3ps = psum.tile([128, 4], FP32, tag="p3ps", name="p3ps")[:, 0:nt]

            # per-slice: X3 mult, then this slice's p2/p3 matmuls (lets X3
            # live in a single buffer and keeps PE fed slice-by-slice)
            prev_ops = None
            nch = len(csl)
            for ci, fs in enumerate(csl):
                if ci == 1 and prev_tree:
                    emit_tree_mid(i - 1)
                if ci == nch - 1 and i >= 1 and (i - 1) in pending:
                    prev_ops = emit_tail(i - 1)
                mul = nc.vector.tensor_mul(X3[:, :, fs, :], X[:, :, fs, :],
                                           X2[:, :, fs, :])
                if prev_ops is not None and ci == nch - 1:
                    bass_add_dep(mul.ins, prev_ops[0].ins, sync=False,
                                 reason="drain prev-tile combine first")
                if mode != "tree":
                    mm_group(bcast_r, p1ps, ident_c1[:], X, fs.stop,
                             f_start=fs.start, start=(fs.start == 0),
                             stop_last=(fs.stop == F))
                mm_group(bcast_r, p2ps, ident_c2[:], X2, fs.stop,
                         f_start=fs.start, start=(fs.start == 0),
                         stop_last=(fs.stop == F))
                mm_group(bcast_rd, p3ps, ident_c3[:], X3, fs.stop,
                         f_start=fs.start, start=(fs.start == 0),
                         stop_last=False, reorder=True)

            pending[i] = (g0, nt, mode, X, G := None, p1ps, p2ps, p3ps)
            g0 += nt

        last = len(TILES) - 1
        if pending[last][2] == "tree":
            emit_tree_l1(last)
            emit_tree_mid(last)
        emit_tail(last)
        nc.sync.dma_start(out[:], stage[:])

    _dedupe_ldweights(nc)
    nc.finalize()
    return nc


_NC_CACHE = None


def _get_nc():
    global _NC_CACHE
    if _NC_CACHE is None:
        _NC_CACHE = build_nc()
    return _NC_CACHE


def run(x: np.ndarray, **spmd_kwargs):
    """Run on 8 cores; returns (out (B,1) fp32, BassKernelResults)."""
    assert x.shape == (B, F, D), x.shape
    x = np.ascontiguousarray(x, dtype=np.float32)
    nc = _get_nc()
    in_maps = [{"x": x[i * B_SHARD:(i + 1) * B_SHARD]} for i in range(N_CORES)]
    res = run_bass_kernel_spmd(nc, in_maps, core_ids=list(range(N_CORES)),
                               **spmd_kwargs)
    outs = []
    for i in range(N_CORES):
        o = res.results[i]["out"]          # [128, GROUPS], out[p, g] = b g*128+p
        outs.append(o.T.reshape(B_SHARD, 1))
    return np.concatenate(outs, axis=0), res


def kernel(x: np.ndarray) -> np.ndarray:
    out, _ = run(x)
    return out


if __name__ == "__main__":
    rng = np.random.default_rng(0)
    x = rng.standard_normal((B, F, D)).astype(np.float32)
    out = kernel(x)
    print("out", out.shape, out.dtype, out[:4, 0])
